# revision 1
# baseline (speedup 1.0000x reference)
"""PointsToVolumes (trilinear point splatting) on 8 TRN2 NeuronCores.

Full inputs -> full output. Sharding: core (b, q) owns output y-rows
[64q, 64q+64) of batch b, i.e. vol[b, :, :, 64q:64q+64, :].

Algorithm per core: points are grouped on host by (z-block, y-cell) into
128-point tiles. For each output row block (zb, Y) = [128 rows = (c, z_lo),
256 cols = x] f32 in PSUM, each contributing point tile adds lhsT.T @ rhs:
  lhsT[k, c*64+zl] = amp_c[k] * wy_dy[k] * tent(zl - z_k)   (bf16, on chip)
  rhs [k, x]       = tent(x - x_k)                          (bf16, on chip)
tent(d) = relu(1 - |d|) reproduces the trilinear weights exactly and drops
out-of-grid corners automatically. Tents are built in batches of BT tiles
with broadcast access patterns, split across GpSimd (diffs), DVE (-|d| via
one scalar_tensor_tensor, lhsT scaling) and ACT (final relu). PSUM blocks
are evicted through SBUF (alternating DVE/ACT) and written to HBM as
contiguous 512KB DMAs alternating between the two HWDGE queues (sync/
scalar). The core-local output layout is [zb, y, c, zl, x]; the host
transposes shards into the final [b, c, z, y, x]. No collectives needed.
"""

import sys
import types

import numpy as np

import concourse.bass as bass
import concourse.mybir as mybir
import concourse.tile as tile

# ---------------------------------------------------------------------------
# Container workarounds (this neuronxcc allows at most 1 sync wait per
# instruction and cannot compile Drain): split waits onto NOPs, skip the
# TileContext tail drain, and register the NTFF profiling hook.
# ---------------------------------------------------------------------------
if "antenv.axon_hooks" not in sys.modules:
    try:
        from trn_agent_boot.trn_boot import _ntff_profile_via_ctypes

        _mod = types.ModuleType("antenv.axon_hooks")
        _hook = _ntff_profile_via_ctypes("/opt/axon/libaxon_pjrt.so")
        _mod.get_axon_ntff_profile_hook = lambda: _hook
        sys.modules["antenv.axon_hooks"] = _mod
    except Exception:
        pass

import concourse.bass_utils as bu  # noqa: E402

bu.upload_artifacts = lambda tmpdir: "local://skipped"


def _nodrain(self, tick_clock, wait_clock):
    self.nc.all_engine_barrier()
    assert self.sems is not None
    popped = self.nc._tile_sem_poison_stack.pop()
    assert popped is self._sem_poison
    self.nc.clear_and_free_semaphores(list(self.sems.allocated().values()))
    self.nc.all_engine_barrier()


tile.TileContext._drain_and_barrier = _nodrain

_MAX_WAITS = 1
_nop_id = [0]


def _split_excess_waits(nc, max_waits=_MAX_WAITS):
    for f in nc.m.functions:
        for bb in f.blocks:
            ins = bb.instructions
            i = 0
            while i < len(ins):
                inst = ins[i]
                si = inst.sync_info
                if si is not None and si.on_wait and len(si.on_wait) > max_waits:
                    waits = list(si.on_wait)
                    excess, keep = waits[:-max_waits], waits[-max_waits:]
                    inst.sync_info = mybir.SyncInfo(
                        on_wait=keep, on_update=list(si.on_update)
                    )
                    while excess:
                        chunk, excess = excess[:max_waits], excess[max_waits:]
                        _nop_id[0] += 1
                        nop = mybir.InstNoOp(
                            name=f"waitnop-{_nop_id[0]}", ins=[], outs=[]
                        )
                        nop.engine = inst.engine
                        nop.sync_info = mybir.SyncInfo(on_wait=chunk, on_update=[])
                        ins.insert(i, nop)
                        i += 1
                i += 1


# ---------------------------------------------------------------------------
# Problem constants (hardcoded per the task contract).
# ---------------------------------------------------------------------------
G = 256          # grid side
NB = 2           # batches
NCH = 2          # amplitude channels
NQ = 4           # y-quarters (cores = NB * NQ = 8)
QH = G // NQ     # 64 y-rows per core
NZB = 4          # z-blocks
ZBH = G // NZB   # 64 z-planes per block
P = 128
BT = 16          # tiles per tent-build batch
DMA_BATCH = 4    # y-row blocks per output DMA (512KB each)
dt = mybir.dt

_AP = mybir.AluOpType
_AF = mybir.ActivationFunctionType


# ---------------------------------------------------------------------------
# Host-side prep
# ---------------------------------------------------------------------------
def _host_prep(positions, amplitudes):
    slots = [(zb, ycl) for zb in range(NZB) for ycl in range(-1, QH)]
    n_slots = len(slots)
    per_core = []
    for b in range(NB):
        p = (positions[b].astype(np.float64) + 0.5) * G
        px, py, pz = (
            p[:, 0].astype(np.float32),
            p[:, 1].astype(np.float32),
            p[:, 2].astype(np.float32),
        )
        amp = amplitudes[b]
        y0 = np.floor(py).astype(np.int64)
        z0 = np.floor(pz).astype(np.int64)
        zb0 = z0 // ZBH
        strad_mask = (z0 % ZBH == ZBH - 1) & (z0 + 1 < G)
        for q in range(NQ):
            ylo, yhi = QH * q - 1, QH * q + QH - 1
            sel = (y0 >= ylo) & (y0 <= yhi)
            idx = np.nonzero(sel)[0]
            sid = idx[strad_mask[idx]]
            ent_pt = np.concatenate([idx, sid])
            ent_zb = np.concatenate([zb0[idx], zb0[sid] + 1])
            ent_yc = y0[ent_pt] - QH * q
            key = ent_zb * (QH + 1) + (ent_yc + 1)
            order = np.argsort(key, kind="stable")
            ent_pt, key = ent_pt[order], key[order]
            counts = np.bincount(key, minlength=n_slots)
            starts = np.concatenate([[0], np.cumsum(counts)])
            per_core.append({
                "pt": ent_pt, "counts": counts, "starts": starts,
                "px": px, "py": py, "pz": pz, "amp": amp, "q": q,
            })

    counts_all = np.stack([c["counts"] for c in per_core])
    ntiles = ((counts_all.max(0) + P - 1) // P).astype(np.int64)
    T = int(ntiles.sum())

    import ml_dtypes
    bf16 = ml_dtypes.bfloat16
    in_maps = []
    for core in per_core:
        NXI = np.full((P, T), 4096.0, np.float32)
        FX = np.zeros((P, T), np.float32)
        NZI = np.full((P, T), 4096.0, np.float32)
        FZ = np.zeros((P, T), np.float32)
        FY = np.zeros((P, T), np.float32)
        A0 = np.zeros((P, T), np.float32)
        A1 = np.zeros((P, T), np.float32)
        tcol = 0
        for si, (zb, ycl) in enumerate(slots):
            nt = int(ntiles[si])
            if nt == 0:
                continue
            s, e = core["starts"][si], core["starts"][si + 1]
            pts = core["pt"][s:e]
            n = len(pts)
            cap = nt * P

            def put(dst, vals, fill):
                col = np.full((cap,), fill, np.float32)
                col[:n] = vals
                dst[:, tcol:tcol + nt] = col.reshape(nt, P).T

            pxi = np.floor(core["px"][pts])
            pzi = np.floor(core["pz"][pts])
            put(NXI, -pxi, 4096.0)
            put(FX, core["px"][pts] - pxi, 0.0)
            put(NZI, ZBH * zb - pzi, 4096.0)
            put(FZ, core["pz"][pts] - pzi, 0.0)
            put(FY, core["py"][pts] - (QH * core["q"] + ycl), 0.0)
            put(A0, core["amp"][0, pts], 0.0)
            put(A1, core["amp"][1, pts], 0.0)
            tcol += nt
        in_maps.append({
            "NXI": NXI.astype(bf16), "FX": FX.astype(bf16),
            "NZI": NZI.astype(bf16), "FZ": FZ.astype(bf16),
            "FY": FY, "A0": A0, "A1": A1,
        })
    return slots, ntiles, T, in_maps


# ---------------------------------------------------------------------------
# Device program
# ---------------------------------------------------------------------------
def _build_program(slots, ntiles, T):
    nc = bass.Bass()
    NXI = nc.declare_dram_parameter("NXI", [P, T], dt.bfloat16, isOutput=False)
    FX = nc.declare_dram_parameter("FX", [P, T], dt.bfloat16, isOutput=False)
    NZI = nc.declare_dram_parameter("NZI", [P, T], dt.bfloat16, isOutput=False)
    FZ = nc.declare_dram_parameter("FZ", [P, T], dt.bfloat16, isOutput=False)
    FY = nc.declare_dram_parameter("FY", [P, T], dt.float32, isOutput=False)
    A0 = nc.declare_dram_parameter("A0", [P, T], dt.float32, isOutput=False)
    A1 = nc.declare_dram_parameter("A1", [P, T], dt.float32, isOutput=False)
    # core-local layout: [zb, y, c, zl, x]; block (zb, y) is contiguous 128KB
    OUT = nc.declare_dram_parameter("OUT", [NZB, QH, NCH, ZBH, G], dt.float32,
                                    isOutput=True)

    with tile.TileContext(nc) as tc:
        with (
            tc.tile_pool(name="const", bufs=1) as cpool,
            tc.tile_pool(name="batch", bufs=1) as bpool,
            tc.tile_pool(name="tents", bufs=2) as tpool,
            tc.tile_pool(name="lhs", bufs=2) as lpool,
            tc.tile_pool(name="stage", bufs=3) as spool,
            tc.tile_pool(name="psum", bufs=4, space="PSUM") as ppool,
        ):
            iota64 = cpool.tile([P, ZBH], dt.bfloat16)
            iota256 = cpool.tile([P, G], dt.bfloat16)
            nc.gpsimd.iota(iota64[:], pattern=[[1, ZBH]], base=0,
                           channel_multiplier=0,
                           allow_small_or_imprecise_dtypes=True)
            nc.gpsimd.iota(iota256[:], pattern=[[1, G]], base=0,
                           channel_multiplier=0,
                           allow_small_or_imprecise_dtypes=True)

            nxi_t = bpool.tile([P, T], dt.bfloat16)
            fx_t = bpool.tile([P, T], dt.bfloat16)
            nzi_t = bpool.tile([P, T], dt.bfloat16)
            fz_t = bpool.tile([P, T], dt.bfloat16)
            fy_t = bpool.tile([P, T], dt.float32)
            a0_t = bpool.tile([P, T], dt.float32)
            a1_t = bpool.tile([P, T], dt.float32)
            for ap, h in ((nxi_t, NXI), (fx_t, FX), (nzi_t, NZI), (fz_t, FZ),
                          (fy_t, FY), (a0_t, A0), (a1_t, A1)):
                nc.sync.dma_start(out=ap[:], in_=h[:])

            w0_t = bpool.tile([P, T], dt.float32)
            sd0_t = bpool.tile([P, T, 2], dt.float32)  # dy0: (a0*w0, a1*w0)
            sd1_t = bpool.tile([P, T, 2], dt.float32)  # dy1: (a0*fy, a1*fy)
            nc.vector.tensor_scalar(out=w0_t[:], in0=fy_t[:], scalar1=-1.0,
                                    scalar2=1.0, op0=_AP.mult, op1=_AP.add)
            nc.vector.tensor_tensor(out=sd0_t[:, :, 0], in0=a0_t[:],
                                    in1=w0_t[:], op=_AP.mult)
            nc.vector.tensor_tensor(out=sd0_t[:, :, 1], in0=a1_t[:],
                                    in1=w0_t[:], op=_AP.mult)
            nc.vector.tensor_tensor(out=sd1_t[:, :, 0], in0=a0_t[:],
                                    in1=fy_t[:], op=_AP.mult)
            nc.vector.tensor_tensor(out=sd1_t[:, :, 1], in0=a1_t[:],
                                    in1=fy_t[:], op=_AP.mult)

            def build_group(g):
                g0 = g * BT
                nb = min(BT, T - g0)
                dx = tpool.tile([P, BT, G], dt.bfloat16, tag="dx", name=f"dx{g}")
                ax = tpool.tile([P, BT, G], dt.bfloat16, tag="ax", name=f"ax{g}")
                ux = tpool.tile([P, BT, G], dt.bfloat16, tag="ux", name=f"ux{g}")
                tx = tpool.tile([P, BT, G], dt.bfloat16, tag="tx", name=f"tx{g}")
                dz = tpool.tile([P, BT, ZBH], dt.bfloat16, tag="dz",
                                name=f"dz{g}")
                az = tpool.tile([P, BT, ZBH], dt.bfloat16, tag="az",
                                name=f"az{g}")
                uz = tpool.tile([P, BT, ZBH], dt.bfloat16, tag="uz",
                                name=f"uz{g}")
                tz = tpool.tile([P, BT, ZBH], dt.bfloat16, tag="tz",
                                name=f"tz{g}")
                lh0 = lpool.tile([P, BT, P], dt.bfloat16, tag="lh0",
                                 name=f"lh0{g}")
                lh1 = lpool.tile([P, BT, P], dt.bfloat16, tag="lh1",
                                 name=f"lh1{g}")

                iox = iota256[:, None, :].to_broadcast([P, nb, G])
                ioz = iota64[:, None, :].to_broadcast([P, nb, ZBH])
                nxiB = nxi_t[:, g0:g0 + nb, None].to_broadcast([P, nb, G])
                fxB = fx_t[:, g0:g0 + nb, None].to_broadcast([P, nb, G])
                nziB = nzi_t[:, g0:g0 + nb, None].to_broadcast([P, nb, ZBH])
                fzB = fz_t[:, g0:g0 + nb, None].to_broadcast([P, nb, ZBH])

                # 2D contiguous views for fast DVE modes
                dx2 = dx[:, 0:nb * G].rearrange("p (b g) -> p (b g)") \
                    if False else dx[:].rearrange("p b g -> p (b g)")[:, :nb * G]
                ax2 = ax[:].rearrange("p b g -> p (b g)")[:, :nb * G]
                ux2 = ux[:].rearrange("p b g -> p (b g)")[:, :nb * G]
                tx2 = tx[:].rearrange("p b g -> p (b g)")[:, :nb * G]
                dz2 = dz[:].rearrange("p b g -> p (b g)")[:, :nb * ZBH]
                az2 = az[:].rearrange("p b g -> p (b g)")[:, :nb * ZBH]
                uz2 = uz[:].rearrange("p b g -> p (b g)")[:, :nb * ZBH]
                tz2 = tz[:].rearrange("p b g -> p (b g)")[:, :nb * ZBH]
                # d = (j - xi) - fx ; tent = min(relu(1 - d), relu(1 + d))
                nc.vector.tensor_tensor(out=dx[:, :nb], in0=iox, in1=nxiB,
                                        op=_AP.add)
                nc.gpsimd.tensor_tensor(out=dx[:, :nb], in0=dx[:, :nb],
                                        in1=fxB, op=_AP.subtract)
                nc.scalar.activation(ax2, dx2, _AF.Relu, bias=1.0, scale=-1.0)
                nc.vector.tensor_scalar(out=ux2, in0=dx2,
                                        scalar1=1.0, scalar2=0.0,
                                        op0=_AP.add, op1=_AP.max)
                nc.vector.tensor_tensor(out=tx2, in0=ax2, in1=ux2, op=_AP.min)
                nc.gpsimd.tensor_tensor(out=dz[:, :nb], in0=ioz, in1=nziB,
                                        op=_AP.add)
                nc.vector.tensor_tensor(out=dz[:, :nb], in0=dz[:, :nb],
                                        in1=fzB, op=_AP.subtract)
                nc.scalar.activation(az2, dz2, _AF.Relu, bias=1.0, scale=-1.0)
                nc.vector.tensor_scalar(out=uz2, in0=dz2,
                                        scalar1=1.0, scalar2=0.0,
                                        op0=_AP.add, op1=_AP.max)
                nc.vector.tensor_tensor(out=tz2, in0=az2, in1=uz2, op=_AP.min)
                # lhsT: one packed op per dy; out contiguous [P, nb, 2, 64]
                tzB = tz[:, 0:nb, None, :].to_broadcast([P, nb, 2, ZBH])
                for dy, lh in ((0, lh0), (1, lh1)):
                    eng = (nc.vector, nc.gpsimd)[dy]
                    sd = (sd0_t, sd1_t)[dy]
                    eng.tensor_tensor(
                        out=lh[:, :nb].rearrange("p b (c z) -> p b c z", c=2),
                        in0=tzB,
                        in1=sd[:, g0:g0 + nb, :, None].to_broadcast(
                            [P, nb, 2, ZBH]),
                        op=_AP.mult)
                return tx, lh0, lh1

            tcol = 0
            cur = build_group(0)
            cur_g = 0
            blocks = {}
            started = set()
            flip = [0]

            def get_tile(t):
                nonlocal cur, cur_g
                g = t // BT
                if g != cur_g:
                    cur = build_group(g)
                    cur_g = g
                j = t - g * BT
                return cur[0][:, j, :], cur[1][:, j, :], cur[2][:, j, :]

            for zbi in range(NZB):
                slot_list = [(si, s) for si, s in enumerate(slots)
                             if s[0] == zbi]
                contrib = {}
                for si, (zb, ycl) in slot_list:
                    nt = int(ntiles[si])
                    if ycl >= 0:
                        contrib[ycl] = contrib.get(ycl, 0) + nt
                    if ycl + 1 < QH:
                        contrib[ycl + 1] = contrib.get(ycl + 1, 0) + nt
                done = {}
                stage = None
                for si, (zb, ycl) in slot_list:
                    nt = int(ntiles[si])
                    for j in range(nt):
                        t = tcol + j
                        tx, lh0, lh1 = get_tile(t)
                        for dy, lh in ((0, lh0), (1, lh1)):
                            Y = ycl + dy
                            if Y < 0 or Y >= QH:
                                continue
                            if Y not in blocks:
                                blocks[Y] = ppool.tile(
                                    [P, G], dt.float32, tag="blk",
                                    name=f"blk{zbi}_{Y}")
                            ps = blocks[Y]
                            first = Y not in started
                            started.add(Y)
                            d = done.get(Y, 0) + 1
                            done[Y] = d
                            nc.tensor.matmul(out=ps[:], lhsT=lh, rhs=tx,
                                             start=first,
                                             stop=(d == contrib[Y]))
                    tcol += nt
                    if ycl >= 0:
                        jb = ycl % DMA_BATCH
                        if jb == 0:
                            stage = spool.tile([P, DMA_BATCH, G], dt.float32,
                                               tag="st",
                                               name=f"st{zbi}_{ycl}")
                        if ycl in blocks:
                            ps = blocks.pop(ycl)
                            started.discard(ycl)
                            if flip[0] % 2 == 0:
                                nc.vector.tensor_copy(out=stage[:, jb, :],
                                                      in_=ps[:])
                            else:
                                nc.scalar.copy(out=stage[:, jb, :], in_=ps[:])
                            flip[0] += 1
                        else:
                            nc.vector.memset(stage[:, jb, :], 0.0)
                        if jb == DMA_BATCH - 1:
                            y0 = ycl - (DMA_BATCH - 1)
                            eng = (nc.sync, nc.scalar)[(ycl // DMA_BATCH) % 2]
                            eng.dma_start(
                                out=OUT[zbi, y0:y0 + DMA_BATCH].rearrange(
                                    "j c z x -> c z j x"),
                                in_=stage[:])
                assert not blocks, (zbi, blocks.keys())
    return nc


_PROGRAM_CACHE = {}


def _append_dma_drain(nc):
    """Synthesize the un-compilable Drain: before kernel end, SP waits for
    every DMA queue semaphore to reach its total increment count, so no DMA
    is still in flight when the NEFF completes."""
    totals = {}
    names = {}
    body_blocks = []
    for f in nc.m.functions:
        for bb in f.blocks:
            body_blocks.append(bb)
            for inst in bb.instructions:
                if inst.opcode != "DMACopy":
                    continue
                si = inst.sync_info
                if not si:
                    continue
                for u in si.on_update:
                    if u.sync_type == "semaphore":
                        totals[u.id] = totals.get(u.id, 0) + u.update_value
                        names[u.id] = u.ant_name
    end_bb = None
    for bb in body_blocks:
        if bb.name.endswith("_end"):
            end_bb = bb
    if end_bb is None or not totals:
        return 0
    pos = 0
    for sem_id, total in sorted(totals.items()):
        _nop_id[0] += 1
        nop = mybir.InstNoOp(name=f"dmadrain-{_nop_id[0]}", ins=[], outs=[])
        nop.engine = mybir.EngineType.SP
        w = mybir.SyncWait(ant_name=names[sem_id], id=sem_id,
                           sync_type="semaphore", wait_mode="sem-ge-imm",
                           wait_value=total)
        nop.sync_info = mybir.SyncInfo(on_wait=[w], on_update=[])
        end_bb.instructions.insert(pos, nop)
        pos += 1
    return len(totals)


def kernel(positions, amplitudes, trace=False, tmpdir=None):
    positions = np.asarray(positions)
    amplitudes = np.asarray(amplitudes)
    slots, ntiles, T, in_maps = _host_prep(positions, amplitudes)

    key = (T, tuple(int(x) for x in ntiles))
    if key not in _PROGRAM_CACHE:
        nc = _build_program(slots, ntiles, T)
        _split_excess_waits(nc)
        _append_dma_drain(nc)
        _PROGRAM_CACHE[key] = nc
    nc = _PROGRAM_CACHE[key]

    core_ids = list(range(NB * NQ))
    res = bu.run_bass_kernel_spmd(nc, in_maps, core_ids, trace=trace,
                                  tmpdir=tmpdir)

    out = np.zeros((NB, NCH, G, G, G), np.float32)
    for cid in core_ids:
        b, q = divmod(cid, NQ)
        # [zb, y, c, zl, x] -> [c, zb*64+zl, y, x]
        co = res.results[cid]["OUT"]
        out[b, :, :, QH * q:QH * q + QH, :] = (
            co.transpose(2, 0, 3, 1, 4).reshape(NCH, G, QH, G))
    if trace:
        kernel.last_exec_ns = res.exec_time_ns
    return out


kernel.last_exec_ns = None

